# revision 14
# baseline (speedup 1.0000x reference)
"""Trainium2 Bass kernel for nn_Attention (cross-attention, B=2 S=2048 D=1024 H=16).

Sharding: 8 cores = data-parallel over batch (2) x tensor-parallel over head
groups (4 groups of 4 heads). Each core computes q/k/v projections for its
256 output dims plus softmax(QK^T)V for its 4 heads; outputs are disjoint
slices of the full output, gathered host-side (no collectives).

On-chip layout avoids all transposes by computing everything in
"transposed" orientation:
  qT/kT [dim, token]  <- W^T stationary, x^T streamed (x^T built host-side)
  scoresT[j, i]       <- kT chunk stationary (K=64), qT streamed
  exp on ScalarE straight out of PSUM (softmax max-subtraction dropped:
    |scores| < ~4 for this problem, exp is safe in fp32)
  outT[c, i] accum    <- [v | ones] stationary, expT streamed; the ones
    column yields the softmax denominator for free, divided out on-chip.
Matmuls use float32r (full-rate fp32 PE mode). Resident tensors are split
into per-chunk tiles so attention on heads 0/1 overlaps the remaining
projections (Tile tracks dependencies per tile).
"""

import numpy as np

import concourse.bass as bass
import concourse.mybir as mybir
import concourse.tile as tile
from concourse.bass_utils import run_bass_kernel_spmd

B, S, D, H = 2, 2048, 1024, 16
HD = D // H  # 64 head dim
N_CORES = 8
HG = 4  # head groups = cores per batch entry
DH = D // HG  # 256 output dims per core
HPC = H // HG  # 4 heads per core
NF = D // 128  # 8 feature (contraction) chunks
F32 = mybir.dt.float32
F32R = mybir.dt.float32r
EXP = mybir.ActivationFunctionType.Exp


def _split_excess_waits(nc, cap=1):
    """This container's walrus caps sync waits at 1/instruction. Hoist excess
    waits onto InstNoOps inserted just before the instruction (same engine)."""
    ctr = 0
    for bb in nc.main_func.blocks:
        insts = list(bb.instructions)
        out = []
        changed = False
        for inst in insts:
            si = inst.sync_info
            waits = list(si.on_wait) if (si is not None and si.on_wait) else []
            if len(waits) > cap:
                changed = True
                for w in waits[:-cap]:
                    ctr += 1
                    out.append(
                        mybir.InstNoOp(
                            name=f"I-waitsplit-{ctr}",
                            sync_info=mybir.SyncInfo(on_wait=[w], on_update=[]),
                            engine=inst.engine,
                            ins=[],
                            outs=[],
                        )
                    )
                inst.sync_info = mybir.SyncInfo(
                    on_wait=waits[-cap:], on_update=list(si.on_update or [])
                )
            out.append(inst)
        if changed:
            bb.instructions = out
    return ctr


def build_nc(s=S, split_waits=True, repeat=1):
    """One core's program (SPMD: all cores run it on their own shard)."""
    nj = s // 128  # j (key token) chunks
    pw = min(1024, s // 2)  # psum block width (i block)
    nih = s // pw  # number of i blocks
    pc = max(min(512, s), DH)  # projection psum chunk width

    nc = bass.Bass()
    xT = nc.dram_tensor("xT", [D, s], F32R, kind="ExternalInput")
    cT = nc.dram_tensor("cT", [D, s], F32R, kind="ExternalInput")
    wqT = nc.dram_tensor("wqT", [D, DH], F32R, kind="ExternalInput")
    wkT = nc.dram_tensor("wkT", [D, DH], F32R, kind="ExternalInput")
    wvT = nc.dram_tensor("wvT", [D, DH], F32R, kind="ExternalInput")
    onesd = nc.dram_tensor("onesd", [128, HPC], F32R, kind="ExternalInput")
    out = nc.dram_tensor("out", [DH, s], F32, kind="ExternalOutput")

    with tile.TileContext(nc) as tc:
        with (
            tc.tile_pool(name="w", bufs=1) as wpool,
            tc.tile_pool(name="stream", bufs=NF) as spool,
            tc.tile_pool(name="res", bufs=1) as rpool,
            tc.tile_pool(name="vabp", bufs=nj) as vpool,
            tc.tile_pool(name="et", bufs=6) as epool,
            tc.tile_pool(name="sm", bufs=2) as smpool,
            tc.tile_pool(name="ps", bufs=2, space="PSUM") as ps,
            tc.tile_pool(name="pj", bufs=1, space="PSUM") as pj,
            tc.tile_pool(name="pv", bufs=1, space="PSUM") as pvp,
            tc.tile_pool(name="dram", bufs=2, space="DRAM") as dpool,
        ):
            # resident weights [feat_part, feat_chunk, outdim]
            wq_sb = wpool.tile([128, NF, DH], F32R, tag="wq")
            wk_sb = wpool.tile([128, NF, DH], F32R, tag="wk")
            wv_sb = wpool.tile([128, NF, DH], F32R, tag="wv")
            nc.sync.dma_start(wq_sb[:], wqT.rearrange("(f p) o -> p f o", p=128))
            nc.sync.dma_start(wk_sb[:], wkT.rearrange("(f p) o -> p f o", p=128))
            nc.sync.dma_start(wv_sb[:], wvT.rearrange("(f p) o -> p f o", p=128))
            ones_sb = wpool.tile([128, HPC], F32R, tag="ones")
            nc.sync.dma_start(ones_sb[:], onesd[:])

            xTr = xT.rearrange("(f p) t -> p f t", p=128)
            cTr = cT.rearrange("(f p) t -> p f t", p=128)

            for _rep in range(repeat):
                # stream x^T per feature chunk
                xt = []
                for f in range(NF):
                    t = spool.tile([128, s], F32R, tag="st")
                    nc.sync.dma_start(t[:], xTr[:, f, :])
                    xt.append(t)

                def proj_o(w_sb, src, o, dst):
                    """dst[:, i] = sum_f w_sb[:,f,o*128:+128]^T @ src_f[:, i]"""
                    for ib in range(s // pw):
                        pq = pj.tile([128, max(pw, DH)], F32, tag="pp")
                        for f in range(NF):
                            lw = w_sb[:, f, o * 128 : (o + 1) * 128]
                            for w0 in range(0, pw, 512):
                                wd = min(512, pw - w0)
                                nc.tensor.matmul(
                                    pq[:, w0 : w0 + wd],
                                    lw,
                                    src[f][:, ib * pw + w0 : ib * pw + w0 + wd],
                                    start=(f == 0),
                                    stop=(f == NF - 1),
                                )
                        nc.vector.tensor_copy(
                            dst[:, ib * pw : (ib + 1) * pw], pq[:, :pw]
                        )

                # Q projections (both o-chunks) while x tiles are resident
                qT = []
                for o in range(2):
                    q_o = rpool.tile([128, s], F32R, tag=f"qT{o}")
                    proj_o(wq_sb, xt, o, q_o)
                    qT.append(q_o)

                # stream context^T (reuses the same slots as x once free)
                ct = []
                for f in range(NF):
                    t = spool.tile([128, s], F32R, tag="st")
                    nc.sync.dma_start(t[:], cTr[:, f, :])
                    ct.append(t)

                # K o-chunk 0 first so heads 0/1 attention can start early
                kT = [None, None]
                k_o0 = rpool.tile([128, s], F32R, tag="kT0", name="k_o0")
                kT[0] = k_o0
                proj_o(wk_sb, ct, 0, kT[0])

                # V projection per j-chunk: v[j, o] = sum_f cT[f,j] * WvT[f,o]
                vab = []
                for jc in range(nj):
                    pvv = pj.tile([128, max(pw, DH)], F32, tag="pp")
                    for f in range(NF):
                        nc.tensor.matmul(
                            pvv[:, :DH],
                            ct[f][:, jc * 128 : (jc + 1) * 128],
                            wv_sb[:, f, :],
                            start=(f == 0),
                            stop=(f == NF - 1),
                        )
                    va = vpool.tile([128, HPC * (HD + 1)], F32R, tag="vab")
                    dst = va.rearrange("p (h c) -> p h c", c=HD + 1)
                    nc.vector.tensor_copy(
                        dst[:, :, :HD],
                        pvv[:, :DH].rearrange("p (h c) -> p h c", c=HD),
                    )
                    nc.vector.tensor_copy(dst[:, :, HD : HD + 1], ones_sb[:, :, None])
                    vab.append(va)

                k_o1 = rpool.tile([128, s], F32R, tag="kT1", name="k_o1")
                kT[1] = k_o1
                proj_o(wk_sb, ct, 1, kT[1])

                # ---- attention per head / i-block ----
                for h in range(HPC):
                    oc, pb = h // 2, (h % 2) * 64
                    for ih in range(nih):
                        ppv = pvp.tile([HD + 1, pw], F32, tag="pv")
                        for jt in range(nj):
                            psc = ps.tile([128, pw], F32, tag="sc")
                            lk = kT[oc][pb : pb + 64, jt * 128 : (jt + 1) * 128]
                            for w0 in range(0, pw, 512):
                                wd = min(512, pw - w0)
                                nc.tensor.matmul(
                                    psc[:, w0 : w0 + wd],
                                    lk,
                                    qT[oc][pb : pb + 64, ih * pw + w0 : ih * pw + w0 + wd],
                                    start=True,
                                    stop=True,
                                )
                            et = epool.tile([128, pw], F32R, tag="et")
                            nc.scalar.activation(et[:], psc[:], EXP)
                            lv = vab[jt][:, h * (HD + 1) : (h + 1) * (HD + 1)]
                            for w0 in range(0, pw, 512):
                                wd = min(512, pw - w0)
                                nc.tensor.matmul(
                                    ppv[:, w0 : w0 + wd],
                                    lv,
                                    et[:, w0 : w0 + wd],
                                    start=(jt == 0),
                                    stop=(jt == nj - 1),
                                )
                        rd = smpool.tile([1, pw], F32, tag="rd")
                        nc.vector.reciprocal(rd[:], ppv[HD : HD + 1, :])
                        rdd = dpool.tile([1, pw], F32, tag="rdd")
                        nc.sync.dma_start(rdd[:], rd[:])
                        rdb = smpool.tile([64, pw], F32, tag="rdb")
                        rsrc = rdd[0, :]
                        bsrc = bass.AP(
                            tensor=rsrc.tensor,
                            offset=rsrc.offset,
                            ap=[[0, 64]] + list(rsrc.ap),
                        )
                        nc.sync.dma_start(rdb[:], bsrc)
                        ob = smpool.tile([64, pw], F32, tag="ob")
                        nc.vector.tensor_mul(ob[:], ppv[:HD, :], rdb[:])
                        nc.sync.dma_start(
                            out[h * HD : (h + 1) * HD, ih * pw : (ih + 1) * pw], ob[:]
                        )

    if split_waits:
        _split_excess_waits(nc)
    return nc


def make_in_maps(x, context, Wq, Wkv, s=S):
    """Host-side shard + layout prep. Core c -> (batch c//HG, head group c%HG)."""
    x = np.asarray(x, dtype=np.float32)
    context = np.asarray(context, dtype=np.float32)
    Wq = np.asarray(Wq, dtype=np.float32)
    Wkv = np.asarray(Wkv, dtype=np.float32)
    scale = np.float32(HD**-0.5)
    in_maps = []
    for core in range(N_CORES):
        b, hg = core // HG, core % HG
        sl = slice(hg * DH, (hg + 1) * DH)
        in_maps.append(
            {
                "xT": np.ascontiguousarray(x[b].T),
                "cT": np.ascontiguousarray(context[b].T),
                "wqT": np.ascontiguousarray(Wq[sl].T * scale),
                "wkT": np.ascontiguousarray(Wkv[sl].T),
                "wvT": np.ascontiguousarray(Wkv[D + hg * DH : D + (hg + 1) * DH].T),
                "onesd": np.ones((128, HPC), dtype=np.float32),
            }
        )
    return in_maps


def gather_out(results, s=S):
    full = np.empty((B, s, D), dtype=np.float32)
    for core in range(N_CORES):
        b, hg = core // HG, core % HG
        full[b, :, hg * DH : (hg + 1) * DH] = results[core]["out"].T
    return full


def kernel(x, context, Wq, Wkv):
    nc = build_nc(S)
    in_maps = make_in_maps(x, context, Wq, Wkv, S)
    res = run_bass_kernel_spmd(nc, in_maps, list(range(N_CORES)))
    return gather_out(res.results, S)
